# revision 1
# baseline (speedup 1.0000x reference)
"""Trainium2 Bass kernel for the (non-standard) MultiHeadAttention module.

Reference math (B=4, N=2048, E=512, H=8):
    q/k/v  = x @ W{q,k,v} + b          # (B, N, E*H)
    split:   head h takes columns h::H  -> per-head (N, E) matrices
    attT_h = (k_h^T @ q_h) * 1/sqrt(N) # (f, e) -- attention over the E axis
    A_h    = exp(attT_h)               # softmax numerator (no max-sub
                                       #  needed, logits are O(+-5))
    s_h[e] = sum_f A_h[f, e]
    out row n' = 4e + r gets  sum_hl (A_h^T/s_h) @ P_h + bp
      for h = 2r + hl  (consequence of the reference's raw
      (B,E,H,N)->(B,N,E*H) reshape before the output projection), where
    P_h    = v_h^T @ Wp_half(hl) + bp/2

Key algebraic refactors (this module attends over the E axis and contracts
over n, so everything collapses into E x E space):
  * Gram matrix  X = x_b^T @ x_b  (E x E, once per core):
      attT_h = Wk_h^T X Wq_h + (Wk_h^T xs) (x) bq_h
               + bk_h (x) (Wq_h^T xs + N bq_h),   xs = colsum(x_b)
    -- eliminates the q/k projections entirely.
  * (A @ v^T) @ Wp == A @ (v^T @ Wp) and
    v_h^T @ Wp_hl == Wv_h^T @ G_hl + bv_h (x) colsum(Wp_hl)  with
    G_hl = x_b^T @ Wp_hl computed once per core -- eliminates the v
    projection and the big P matmuls.
  * bp/2 folded into each P_h; softmax normalization at the very end:
    out = U0*r0 + U1*r1,  U_h = A_h^T @ P_h,  r_h = 1/s_h.
  Net: ~7.5 GFLOP and ~560 matmuls per core vs ~26 GFLOP naively.

Sharding: 16 independent units (b, r), b in 0..3, r in 0..3; unit (b, r)
owns heads {2r, 2r+1} and produces output rows out[b, r::4, :].  Two units
per core, batch-major:  core c -> b = c//2, r in {2*(c%2), 2*(c%2)+1}.
No inter-core communication.

All matmuls run as float32r (fp32 storage, reduced-precision single-pass
PE mode: full speed for moving-free-dim >= 256).
"""

import numpy as np
from contextlib import ExitStack

import concourse.bass as bass
import concourse.mybir as mybir
import concourse.tile as tile
from concourse import bacc
from concourse.bass_utils import run_bass_kernel_spmd

B, N, E, H = 4, 2048, 512, 8
NT = N // 128          # 16 contraction chunks of 128 over n
EB = E // 128          # 4 blocks of 128 over e/f
SCALE = float(1.0 / np.sqrt(np.float32(N)))
F32 = mybir.dt.float32
F32R = mybir.dt.float32r
PSUM = bass.MemorySpace.PSUM

_CACHED_NC = None


def _bcast128(ap_nd):
    """DMA access pattern replicating a DRAM region across 128 partitions."""
    return bass.AP(
        tensor=ap_nd.tensor, offset=ap_nd.offset, ap=[[0, 128]] + list(ap_nd.ap)
    )


def build_nc():
    nc = bacc.Bacc("TRN2", target_bir_lowering=False, debug=False)

    xn_d = nc.dram_tensor("xn", (N, E), F32R, kind="ExternalInput")
    wq_d = nc.dram_tensor("wq", (2, 2, 128, EB, E), F32R, kind="ExternalInput")
    wk_d = nc.dram_tensor("wk", (2, 2, 128, EB, E), F32R, kind="ExternalInput")
    wv_d = nc.dram_tensor("wv", (2, 2, 128, EB, E), F32R, kind="ExternalInput")
    wp_d = nc.dram_tensor("wp", (2, N, E), F32R, kind="ExternalInput")
    swp_d = nc.dram_tensor("swp", (1, 2, E), F32R, kind="ExternalInput")
    hvec_d = nc.dram_tensor("hvec", (1, 2, 2, 2, E), F32R, kind="ExternalInput")
    bqkv_d = nc.dram_tensor("bqkv", (2, 2, 1, 3, E), F32R, kind="ExternalInput")
    bph_d = nc.dram_tensor("bph", (E,), F32, kind="ExternalInput")
    ones_d = nc.dram_tensor("ones", (128, 2), F32R, kind="ExternalInput")
    out_d = nc.dram_tensor("out", (2, E, E), F32, kind="ExternalOutput")

    with tile.TileContext(nc) as tc, ExitStack() as ctx:
        consts = ctx.enter_context(tc.tile_pool(name="consts", bufs=1))
        stream = ctx.enter_context(tc.tile_pool(name="stream", bufs=4))
        wqkv_pool = ctx.enter_context(tc.tile_pool(name="wqkv", bufs=2))
        bias_pool = ctx.enter_context(tc.tile_pool(name="bias", bufs=2))
        t1_pool = ctx.enter_context(tc.tile_pool(name="t1", bufs=1))
        a_pool = ctx.enter_context(tc.tile_pool(name="a", bufs=2))
        p_pool = ctx.enter_context(tc.tile_pool(name="p", bufs=2))
        o_pool = ctx.enter_context(tc.tile_pool(name="o", bufs=4))
        r_pool = ctx.enter_context(tc.tile_pool(name="r", bufs=2))
        mm_ps = ctx.enter_context(tc.tile_pool(name="mmps", bufs=2, space=PSUM))
        big_ps = ctx.enter_context(tc.tile_pool(name="bigps", bufs=1, space=PSUM))
        u_ps = ctx.enter_context(tc.tile_pool(name="ups", bufs=2, space=PSUM))

        # Prime the G-phase wp streams (first chunks of each half) before
        # anything else on their queues.
        wp_primed = {}
        for pn in range(3):
            psl = slice(pn * 128, (pn + 1) * 128)
            w0 = stream.tile([128, E], F32R, tag="wp0", name=f"wp0p{pn}")
            nc.gpsimd.dma_start(out=w0[:], in_=wp_d.ap()[0, psl, :])
            w1 = stream.tile([128, E], F32R, tag="wp1", name=f"wp1p{pn}", bufs=6)
            nc.scalar.dma_start(out=w1[:], in_=wp_d.ap()[1, psl, :])
            wp_primed[pn] = (w0, w1)

        # x (natural layout), resident: feeds both the X and G phases.
        # The X phase only gates on these.
        xn_sb = []
        for n in range(NT):
            t = consts.tile([128, E], F32R, tag=f"xn{n}", name=f"xn{n}")
            eng = nc.sync if n % 2 == 0 else nc.scalar
            eng.dma_start(out=t[:], in_=xn_d.ap()[n * 128 : (n + 1) * 128, :])
            xn_sb.append(t)

        # ---- other resident constants ----
        ones_sb = consts.tile([128, 2], F32R, tag="ones")
        nc.gpsimd.dma_start(out=ones_sb[:], in_=ones_d.ap())
        bph_sb = consts.tile([128, E], F32, tag="bph")
        nc.scalar.dma_start(out=bph_sb[:], in_=_bcast128(bph_d.ap()))
        swp_sb = consts.tile([1, 2, E], F32R, tag="swp")
        nc.gpsimd.dma_start(out=swp_sb[:], in_=swp_d.ap())
        hvec_sb = consts.tile([1, 2, 2, 2, E], F32R, tag="hvec")
        nc.scalar.dma_start(out=hvec_sb[:], in_=hvec_d.ap())

        # ---- pass 1: X = x^T x (big: 4 banks) + G0 = x^T Wp0 (mm+u: 4
        # banks), one shared sweep over n so X is not xn-starved alone ----
        X_ps = big_ps.tile([128, EB, E], F32, tag="big")
        g_sb = [
            consts.tile([128, EB, E], F32R, tag=f"g{hl}", name=f"g{hl}")
            for hl in range(2)
        ]
        g0_slots = [
            mm_ps.tile([128, E], F32, tag="mm", name="g0a"),
            mm_ps.tile([128, E], F32, tag="mm", name="g0b"),
            u_ps.tile([128, E], F32, tag="u", name="g0c"),
            u_ps.tile([128, E], F32, tag="u", name="g0d"),
        ]
        gate_gearly = None
        for n in range(NT):
            nsl = slice(n * 128, (n + 1) * 128)
            if n in wp_primed:
                wp0_sb, _ = wp_primed[n]
            else:
                wp0_sb = stream.tile([128, E], F32R, tag="wp0")
                nc.gpsimd.dma_start(out=wp0_sb[:], in_=wp_d.ap()[0, nsl, :])
            for m in range(EB):
                msl = slice(m * 128, (m + 1) * 128)
                nc.tensor.matmul(
                    X_ps[:, m, :],
                    xn_sb[n][:, msl],
                    xn_sb[n][:],
                    start=n == 0,
                    stop=n == NT - 1,
                )
                g_bi = nc.tensor.matmul(
                    g0_slots[m][:],
                    xn_sb[n][:, msl],
                    wp0_sb[:],
                    start=n == 0,
                    stop=n == NT - 1,
                )
                if n == 2 and m == 0:
                    gate_gearly = g_bi.ins
        X_sb = consts.tile([128, EB, E], F32R, tag="X")
        for m in range(EB):
            nc.vector.tensor_copy(X_sb[:, m, :], X_ps[:, m, :])
            nc.vector.tensor_copy(g_sb[0][:, m, :], g0_slots[m][:])

        # ---- pass 2: G1 = x^T Wp1 (mm+u slots again) ----
        g1_slots = [
            mm_ps.tile([128, E], F32, tag="mm", name="g1a"),
            mm_ps.tile([128, E], F32, tag="mm", name="g1b"),
            u_ps.tile([128, E], F32, tag="u", name="g1c"),
            u_ps.tile([128, E], F32, tag="u", name="g1d"),
        ]
        gate_gmid = None
        for n in range(NT):
            nsl = slice(n * 128, (n + 1) * 128)
            if n in wp_primed:
                _, wp1_sb = wp_primed[n]
            else:
                wp1_sb = stream.tile([128, E], F32R, tag="wp1", bufs=6)
                nc.scalar.dma_start(out=wp1_sb[:], in_=wp_d.ap()[1, nsl, :])
            for m in range(EB):
                msl = slice(m * 128, (m + 1) * 128)
                g_bi = nc.tensor.matmul(
                    g1_slots[m][:],
                    xn_sb[n][:, msl],
                    wp1_sb[:],
                    start=n == 0,
                    stop=n == NT - 1,
                )
                if n == NT // 2 and m == 0:
                    gate_gmid = g_bi.ins
        for m in range(EB):
            nc.vector.tensor_copy(g_sb[1][:, m, :], g1_slots[m][:])

        gate_hist = [gate_gearly, gate_gearly]  # per-head early gates
        pending_s = None

        def emit_pending_s():
            nonlocal pending_s
            if pending_s is None:
                return
            A_sb, R_list = pending_s
            pending_s = None
            s_ps = mm_ps.tile([128, EB, 2], F32, tag="mm")
            for eb in range(EB):
                esl = slice(eb * 128, (eb + 1) * 128)
                for fc in range(EB):
                    nc.tensor.matmul(
                        s_ps[:, eb, :],
                        A_sb[:, fc, esl],
                        ones_sb[:],
                        start=fc == 0,
                        stop=fc == EB - 1,
                    )
            r_sb = r_pool.tile([128, EB, 2], F32, tag="r")
            nc.vector.reciprocal(out=r_sb[:], in_=s_ps[:])
            R_list.append(r_sb)

        for u in range(2):
            A_tiles, P_tiles, R_tiles = [], [], []
            for hl in range(2):
                # --- weights + biases for head (u, hl), prefetch-gated ---
                bias_sb = bias_pool.tile([1, 3, E], F32R, tag="bias")
                bias_bi = nc.scalar.dma_start(
                    out=bias_sb[:], in_=bqkv_d.ap()[u, hl]
                )
                wv_sb = wqkv_pool.tile([128, EB, E], F32R, tag="wv")
                wv_bi = nc.gpsimd.dma_start(out=wv_sb[:], in_=wv_d.ap()[u, hl])
                wq_sb = wqkv_pool.tile([128, EB, E], F32R, tag="wq")
                wq_bi = nc.gpsimd.dma_start(out=wq_sb[:], in_=wq_d.ap()[u, hl])
                wk_sb = wqkv_pool.tile([128, EB, E], F32R, tag="wk")
                wk_bi = nc.scalar.dma_start(out=wk_sb[:], in_=wk_d.ap()[u, hl])
                gate = gate_hist[-2]  # two head-phases back
                for bi in (bias_bi, wv_bi, wq_bi, wk_bi):
                    tile.add_dep_helper(bi.ins, gate, reason="delay prefetch")

                # --- P_h = Wv_h^T @ G_hl + bv_h (x) swp_hl + bp/2 ---
                # (independent of the attention path; fills the PE while the
                #  previous head's exp runs on ACT)
                P_sb = p_pool.tile([128, EB, E], F32R, tag="p")

                def emit_p_group(fb):
                    fsl = slice(fb * 128, (fb + 1) * 128)
                    p_ps = u_ps.tile([128, E], F32, tag="u", name=f"pp{fb}")
                    first = None
                    for ec in range(EB):
                        bi = nc.tensor.matmul(
                            p_ps[:],
                            wv_sb[:, ec, fsl],
                            g_sb[hl][:, ec, :],
                            start=ec == 0,
                            stop=False,
                        )
                        first = first or bi
                    nc.tensor.matmul(
                        p_ps[:],
                        bias_sb[0:1, 2, fsl],
                        swp_sb[0:1, hl, :],
                        start=False,
                        stop=True,
                    )
                    nc.vector.tensor_add(P_sb[:, fb, :], p_ps[:], bph_sb[:])
                    return first

                # first half of P (covers the previous head's exp wait)
                p_first = emit_p_group(0)
                emit_p_group(1)
                gate_early = p_first.ins

                # --- T1 = X @ Wq_h (uses X symmetry: lhsT = X slices) ---
                T1_ps = big_ps.tile([128, EB, E], F32, tag="big")
                t1_first = None
                for m in range(EB):
                    msl = slice(m * 128, (m + 1) * 128)
                    for ec in range(EB):
                        bi = nc.tensor.matmul(
                            T1_ps[:, m, :],
                            X_sb[:, ec, msl],
                            wq_sb[:, ec, :],
                            start=ec == 0,
                            stop=ec == EB - 1,
                        )
                        t1_first = t1_first or bi
                gate_mid = t1_first.ins
                T1_sb = t1_pool.tile([128, EB, E], F32R, tag="t1")
                for m in range(EB):
                    nc.vector.tensor_copy(T1_sb[:, m, :], T1_ps[:, m, :])

                # second half of P + previous head's s: both independent of
                # T1, they cover the T1 psum->sbuf copy before attT
                emit_p_group(2)
                emit_p_group(3)
                P_tiles.append(P_sb)
                emit_pending_s()

                # --- attT = Wk_h^T @ T1 + uvec (x) bq + bk (x) wvec ---
                attT_ps = big_ps.tile([128, EB, E], F32, tag="big")
                for fb in range(EB):
                    fsl = slice(fb * 128, (fb + 1) * 128)
                    for ec in range(EB):
                        nc.tensor.matmul(
                            attT_ps[:, fb, :],
                            wk_sb[:, ec, fsl],
                            T1_sb[:, ec, :],
                            start=ec == 0,
                            stop=False,
                        )
                    nc.tensor.matmul(
                        attT_ps[:, fb, :],
                        hvec_sb[0:1, u, hl, 0, fsl],
                        bias_sb[0:1, 0, :],
                        start=False,
                        stop=False,
                    )
                    nc.tensor.matmul(
                        attT_ps[:, fb, :],
                        bias_sb[0:1, 1, fsl],
                        hvec_sb[0:1, u, hl, 1, :],
                        start=False,
                        stop=True,
                    )

                # --- exp (softmax numerator, transposed layout) ---
                A_sb = a_pool.tile([128, EB, E], F32R, tag="a")
                for fb in range(EB):
                    nc.scalar.activation(
                        out=A_sb[:, fb, :],
                        in_=attT_ps[:, fb, :],
                        func=mybir.ActivationFunctionType.Exp,
                        scale=SCALE,
                    )
                A_tiles.append(A_sb)
                pending_s = (A_sb, R_tiles)
                gate_hist.append(gate_early)

            # --- U_h = A_h^T @ P_h ; out = U0*r0 + U1*r1 ---
            out_tiles = [
                o_pool.tile([128, E], F32, tag="o", name=f"ot{u}_{i}")
                for i in range(EB)
            ]
            for hl in range(2):
                if hl == 1:
                    emit_pending_s()  # s of this unit's second head
                for eb in range(EB):
                    esl = slice(eb * 128, (eb + 1) * 128)
                    u_tile = u_ps.tile([128, E], F32, tag="u")
                    for fc in range(EB):
                        nc.tensor.matmul(
                            u_tile[:],
                            A_tiles[hl][:, fc, esl],
                            P_tiles[hl][:, fc, :],
                            start=fc == 0,
                            stop=fc == EB - 1,
                        )
                    if hl == 0:
                        nc.vector.tensor_scalar_mul(
                            out_tiles[eb][:], u_tile[:], R_tiles[0][:, eb, 0:1]
                        )
                    else:
                        nc.vector.scalar_tensor_tensor(
                            out_tiles[eb][:],
                            u_tile[:],
                            R_tiles[1][:, eb, 0:1],
                            out_tiles[eb][:],
                            op0=mybir.AluOpType.mult,
                            op1=mybir.AluOpType.add,
                        )
                    if hl == 1:
                        nc.sync.dma_start(
                            out=out_d.ap()[u, eb * 128 : (eb + 1) * 128, :],
                            in_=out_tiles[eb][:],
                        )

    nc.compile()
    return nc


def _get_nc():
    global _CACHED_NC
    if _CACHED_NC is None:
        _CACHED_NC = build_nc()
    return _CACHED_NC


def make_in_maps(x, Wq, bq, Wk, bk, Wv, bv, Wp, bp):
    x = np.asarray(x, np.float32)
    Wq, Wk, Wv, Wp = (np.asarray(a, np.float32) for a in (Wq, Wk, Wv, Wp))
    bq, bk, bv, bp = (np.asarray(a, np.float32) for a in (bq, bk, bv, bp))
    wp_arr = np.ascontiguousarray(np.stack([Wp[:N], Wp[N:]]))
    swp = np.ascontiguousarray(np.stack([Wp[:N].sum(0), Wp[N:].sum(0)])[None])
    bph = np.ascontiguousarray(0.5 * bp)
    in_maps = []
    for c in range(8):
        b = c // 2
        rs = [2 * (c % 2), 2 * (c % 2) + 1]
        heads = [[2 * r + hl for hl in range(2)] for r in rs]
        xs = x[b].sum(0)

        def tile_w(Wm, h):
            # (E, E) -> [p, t, e] with row t*128+p on partition p
            return Wm[:, h::H].reshape(EB, 128, E).transpose(1, 0, 2)

        wq_arr = np.ascontiguousarray(
            np.stack([[tile_w(Wq, h) for h in hu] for hu in heads])
        )
        wk_arr = np.ascontiguousarray(
            np.stack([[tile_w(Wk, h) for h in hu] for hu in heads])
        )
        wv_arr = np.ascontiguousarray(
            np.stack([[tile_w(Wv, h) for h in hu] for hu in heads])
        )
        bqkv = np.ascontiguousarray(
            np.stack([[[bq[h::H], bk[h::H], bv[h::H]] for h in hu] for hu in heads])[
                :, :, None
            ]
        )
        hvec = np.ascontiguousarray(
            np.stack(
                [
                    [
                        [
                            Wk[:, h::H].T @ xs,
                            Wq[:, h::H].T @ xs + np.float32(N) * bq[h::H],
                        ]
                        for h in hu
                    ]
                    for hu in heads
                ]
            )[None]
        )
        in_maps.append(
            {
                "xn": np.ascontiguousarray(x[b]),
                "wq": wq_arr,
                "wk": wk_arr,
                "wv": wv_arr,
                "wp": wp_arr,
                "swp": swp,
                "hvec": hvec,
                "bqkv": bqkv,
                "bph": bph,
                "ones": np.ones((128, 2), np.float32),
            }
        )
    return in_maps


def assemble_out(results):
    out = np.empty((B, N, E), np.float32)
    for c in range(8):
        b = c // 2
        for ui in range(2):
            r = 2 * (c % 2) + ui
            out[b, r::4, :] = results[c]["out"][ui]
    return out


def run(inputs, trace=False, **spmd_kwargs):
    """Full pipeline; returns (output, BassKernelResults)."""
    nc = _get_nc()
    in_maps = make_in_maps(**inputs)
    res = run_bass_kernel_spmd(
        nc, in_maps, core_ids=list(range(8)), trace=trace, **spmd_kwargs
    )
    return assemble_out(res.results), res


def kernel(**inputs):
    out, _ = run(inputs)
    return out



# revision 8
# speedup vs baseline: 1.3665x; 1.3665x over previous
"""Trainium2 Bass kernel for the (non-standard) MultiHeadAttention module.

Reference math (B=4, N=2048, E=512, H=8):
    q/k/v  = x @ W{q,k,v} + b          # (B, N, E*H)
    split:   head h takes columns h::H  -> per-head (N, E) matrices
    attT_h = (k_h^T @ q_h) * 1/sqrt(N) # (f, e) -- attention over the E axis
    A_h    = exp(attT_h)               # softmax numerator (no max-sub
                                       #  needed, logits are O(+-5))
    s_h[e] = sum_f A_h[f, e]
    out row n' = 4e + r gets  sum_hl (A_h^T/s_h) @ P_h + bp
      for h = 2r + hl  (consequence of the reference's raw
      (B,E,H,N)->(B,N,E*H) reshape before the output projection), where
    P_h    = v_h^T @ Wp_half(hl) + bp/2

Key algebraic refactors (this module attends over the E axis and contracts
over n, so everything collapses into E x E space):
  * Gram matrix  X = x_b^T @ x_b  (E x E, once per core):
      attT_h = Wk_h^T X Wq_h + (Wk_h^T xs) (x) bq_h
               + bk_h (x) (Wq_h^T xs + N bq_h),   xs = colsum(x_b)
    -- eliminates the q/k projections entirely.
  * (A @ v^T) @ Wp == A @ (v^T @ Wp) and
    v_h^T @ Wp_hl == Wv_h^T @ G_hl + bv_h (x) colsum(Wp_hl)  with
    G_hl = x_b^T @ Wp_hl computed once per core -- eliminates the v
    projection and the big P matmuls.
  * bp/2 folded into each P_h; softmax normalization at the very end:
    out = U0*r0 + U1*r1,  U_h = A_h^T @ P_h,  r_h = 1/s_h.
  Net: ~7.5 GFLOP and ~560 matmuls per core vs ~26 GFLOP naively.

Sharding: 16 independent units (b, r), b in 0..3, r in 0..3; unit (b, r)
owns heads {2r, 2r+1} and produces output rows out[b, r::4, :].  Two units
per core, batch-major:  core c -> b = c//2, r in {2*(c%2), 2*(c%2)+1}.
No inter-core communication.

All matmuls run in bf16 (f32 PSUM accumulation): fp32/f32r moving
operands stream at half rate on real TRN2 HW (2KB/row vs the 1KB/cycle
xbus), so bf16 halves both PE time and HBM traffic. The two rank-1 bias
updates of attT are merged into one contraction-2 matmul (rk1 tensor).
"""

import numpy as np
import ml_dtypes
from contextlib import ExitStack

import concourse.bass as bass
import concourse.mybir as mybir
import concourse.tile as tile
from concourse import bacc
from concourse.bass_utils import run_bass_kernel_spmd

B, N, E, H = 4, 2048, 512, 8
NT = N // 128          # 16 contraction chunks of 128 over n
EB = E // 128          # 4 blocks of 128 over e/f
SCALE = float(1.0 / np.sqrt(np.float32(N)))
F32 = mybir.dt.float32
BF16 = mybir.dt.bfloat16
NP_BF16 = ml_dtypes.bfloat16
PSUM = bass.MemorySpace.PSUM

_CACHED_NC = None


def _bcast128(ap_nd):
    """DMA access pattern replicating a DRAM region across 128 partitions."""
    return bass.AP(
        tensor=ap_nd.tensor, offset=ap_nd.offset, ap=[[0, 128]] + list(ap_nd.ap)
    )


def build_nc():
    nc = bacc.Bacc("TRN2", target_bir_lowering=False, debug=False)

    xn_d = nc.dram_tensor("xn", (N, E), BF16, kind="ExternalInput")
    wq_d = nc.dram_tensor("wq", (2, 2, 128, EB, E), BF16, kind="ExternalInput")
    wk_d = nc.dram_tensor("wk", (2, 2, 128, EB, E), BF16, kind="ExternalInput")
    wv_d = nc.dram_tensor("wv", (2, 2, 128, EB, E), BF16, kind="ExternalInput")
    wp_d = nc.dram_tensor("wp", (2, N, E), BF16, kind="ExternalInput")
    swp_d = nc.dram_tensor("swp", (1, 2, E), BF16, kind="ExternalInput")
    rk1_d = nc.dram_tensor("rk1", (2, 2, 2, 2, E), BF16, kind="ExternalInput")
    bqkv_d = nc.dram_tensor("bqkv", (2, 2, 1, 3, E), BF16, kind="ExternalInput")
    bph_d = nc.dram_tensor("bph", (E,), F32, kind="ExternalInput")
    ones_d = nc.dram_tensor("ones", (128, 2), BF16, kind="ExternalInput")
    out_d = nc.dram_tensor("out", (2, E, E), F32, kind="ExternalOutput")

    with tile.TileContext(nc) as tc, ExitStack() as ctx:
        consts = ctx.enter_context(tc.tile_pool(name="consts", bufs=1))
        stream = ctx.enter_context(tc.tile_pool(name="stream", bufs=4))
        wqkv_pool = ctx.enter_context(tc.tile_pool(name="wqkv", bufs=2))
        bias_pool = ctx.enter_context(tc.tile_pool(name="bias", bufs=2))
        t1_pool = ctx.enter_context(tc.tile_pool(name="t1", bufs=1))
        a_pool = ctx.enter_context(tc.tile_pool(name="a", bufs=2))
        p_pool = ctx.enter_context(tc.tile_pool(name="p", bufs=2))
        o_pool = ctx.enter_context(tc.tile_pool(name="o", bufs=4))
        r_pool = ctx.enter_context(tc.tile_pool(name="r", bufs=2))
        mm_ps = ctx.enter_context(tc.tile_pool(name="mmps", bufs=2, space=PSUM))
        big_ps = ctx.enter_context(tc.tile_pool(name="bigps", bufs=1, space=PSUM))
        u_ps = ctx.enter_context(tc.tile_pool(name="ups", bufs=2, space=PSUM))

        # Prime the G-phase wp streams (first chunks of each half) before
        # anything else on their queues.
        wp_primed = {}
        for pn in range(3):
            psl = slice(pn * 128, (pn + 1) * 128)
            w0 = stream.tile([128, E], BF16, tag="wp0", name=f"wp0p{pn}")
            nc.gpsimd.dma_start(out=w0[:], in_=wp_d.ap()[0, psl, :])
            w1 = stream.tile([128, E], BF16, tag="wp1", name=f"wp1p{pn}", bufs=6)
            nc.scalar.dma_start(out=w1[:], in_=wp_d.ap()[1, psl, :])
            wp_primed[pn] = (w0, w1)

        # x (natural layout), resident: feeds both the X and G phases.
        # The X phase only gates on these.
        xn_sb = []
        for n in range(NT):
            t = consts.tile([128, E], BF16, tag=f"xn{n}", name=f"xn{n}")
            eng = nc.sync if n % 2 == 0 else nc.scalar
            eng.dma_start(out=t[:], in_=xn_d.ap()[n * 128 : (n + 1) * 128, :])
            xn_sb.append(t)

        # ---- other resident constants ----
        ones_sb = consts.tile([128, 2], BF16, tag="ones")
        nc.gpsimd.dma_start(out=ones_sb[:], in_=ones_d.ap())
        bph_sb = consts.tile([128, E], F32, tag="bph")
        nc.scalar.dma_start(out=bph_sb[:], in_=_bcast128(bph_d.ap()))
        swp_sb = consts.tile([1, 2, E], BF16, tag="swp")
        nc.gpsimd.dma_start(out=swp_sb[:], in_=swp_d.ap())
        rk1_sb = consts.tile([2, 2, 2, 2, E], BF16, tag="rk1")
        nc.scalar.dma_start(out=rk1_sb[:], in_=rk1_d.ap())

        # ---- pass 1: X = x^T x (big: 4 banks) + G0 = x^T Wp0 (mm+u: 4
        # banks), one shared sweep over n so X is not xn-starved alone ----
        X_ps = big_ps.tile([128, EB, E], F32, tag="big")
        g_sb = [
            consts.tile([128, EB, E], BF16, tag=f"g{hl}", name=f"g{hl}")
            for hl in range(2)
        ]
        g0_slots = [
            mm_ps.tile([128, E], F32, tag="mm", name="g0a"),
            mm_ps.tile([128, E], F32, tag="mm", name="g0b"),
            u_ps.tile([128, E], F32, tag="u", name="g0c"),
            u_ps.tile([128, E], F32, tag="u", name="g0d"),
        ]
        gate_gearly = None
        for n in range(NT):
            nsl = slice(n * 128, (n + 1) * 128)
            if n in wp_primed:
                wp0_sb, _ = wp_primed[n]
            else:
                wp0_sb = stream.tile([128, E], BF16, tag="wp0")
                nc.gpsimd.dma_start(out=wp0_sb[:], in_=wp_d.ap()[0, nsl, :])
            for m in range(EB):
                msl = slice(m * 128, (m + 1) * 128)
                nc.tensor.matmul(
                    X_ps[:, m, :],
                    xn_sb[n][:, msl],
                    xn_sb[n][:],
                    start=n == 0,
                    stop=n == NT - 1,
                )
                g_bi = nc.tensor.matmul(
                    g0_slots[m][:],
                    xn_sb[n][:, msl],
                    wp0_sb[:],
                    start=n == 0,
                    stop=n == NT - 1,
                )
                if n == 2 and m == 0:
                    gate_gearly = g_bi.ins
        X_sb = consts.tile([128, EB, E], BF16, tag="X")
        for m in range(EB):
            nc.vector.tensor_copy(X_sb[:, m, :], X_ps[:, m, :])
            nc.vector.tensor_copy(g_sb[0][:, m, :], g0_slots[m][:])

        # ---- pass 2: G1 = x^T Wp1 (mm+u slots again) ----
        g1_slots = [
            mm_ps.tile([128, E], F32, tag="mm", name="g1a"),
            mm_ps.tile([128, E], F32, tag="mm", name="g1b"),
            u_ps.tile([128, E], F32, tag="u", name="g1c"),
            u_ps.tile([128, E], F32, tag="u", name="g1d"),
        ]
        gate_gmid = None
        for n in range(NT):
            nsl = slice(n * 128, (n + 1) * 128)
            if n in wp_primed:
                _, wp1_sb = wp_primed[n]
            else:
                wp1_sb = stream.tile([128, E], BF16, tag="wp1", bufs=6)
                nc.scalar.dma_start(out=wp1_sb[:], in_=wp_d.ap()[1, nsl, :])
            for m in range(EB):
                msl = slice(m * 128, (m + 1) * 128)
                g_bi = nc.tensor.matmul(
                    g1_slots[m][:],
                    xn_sb[n][:, msl],
                    wp1_sb[:],
                    start=n == 0,
                    stop=n == NT - 1,
                )
                if n == NT // 2 and m == 0:
                    gate_gmid = g_bi.ins
        for m in range(EB):
            nc.vector.tensor_copy(g_sb[1][:, m, :], g1_slots[m][:])

        gate_hist = [gate_gearly, gate_gearly]  # per-head early gates
        pending_s = None

        def emit_pending_s():
            nonlocal pending_s
            if pending_s is None:
                return
            A_sb, R_list = pending_s
            pending_s = None
            s_ps = mm_ps.tile([128, EB, 2], F32, tag="mm")
            for eb in range(EB):
                esl = slice(eb * 128, (eb + 1) * 128)
                for fc in range(EB):
                    nc.tensor.matmul(
                        s_ps[:, eb, :],
                        A_sb[:, fc, esl],
                        ones_sb[:],
                        start=fc == 0,
                        stop=fc == EB - 1,
                    )
            r_sb = r_pool.tile([128, EB, 2], F32, tag="r")
            nc.vector.reciprocal(out=r_sb[:], in_=s_ps[:])
            R_list.append(r_sb)

        for u in range(2):
            A_tiles, P_tiles, R_tiles = [], [], []
            for hl in range(2):
                # --- weights + biases for head (u, hl), prefetch-gated ---
                bias_sb = bias_pool.tile([1, 3, E], BF16, tag="bias")
                bias_bi = nc.scalar.dma_start(
                    out=bias_sb[:], in_=bqkv_d.ap()[u, hl]
                )
                wv_sb = wqkv_pool.tile([128, EB, E], BF16, tag="wv")
                wv_bi = nc.gpsimd.dma_start(out=wv_sb[:], in_=wv_d.ap()[u, hl])
                wq_sb = wqkv_pool.tile([128, EB, E], BF16, tag="wq")
                wq_bi = nc.gpsimd.dma_start(out=wq_sb[:], in_=wq_d.ap()[u, hl])
                wk_sb = wqkv_pool.tile([128, EB, E], BF16, tag="wk")
                wk_bi = nc.scalar.dma_start(out=wk_sb[:], in_=wk_d.ap()[u, hl])
                gate = gate_hist[-2]  # two head-phases back
                for bi in (bias_bi, wv_bi, wq_bi, wk_bi):
                    tile.add_dep_helper(bi.ins, gate, reason="delay prefetch")

                # --- P_h = Wv_h^T @ G_hl + bv_h (x) swp_hl + bp/2 ---
                # (independent of the attention path; fills the PE while the
                #  previous head's exp runs on ACT)
                P_sb = p_pool.tile([128, EB, E], BF16, tag="p")

                def emit_p_group(fb):
                    fsl = slice(fb * 128, (fb + 1) * 128)
                    p_ps = u_ps.tile([128, E], F32, tag="u", name=f"pp{fb}")
                    first = None
                    for ec in range(EB):
                        bi = nc.tensor.matmul(
                            p_ps[:],
                            wv_sb[:, ec, fsl],
                            g_sb[hl][:, ec, :],
                            start=ec == 0,
                            stop=False,
                        )
                        first = first or bi
                    nc.tensor.matmul(
                        p_ps[:],
                        bias_sb[0:1, 2, fsl],
                        swp_sb[0:1, hl, :],
                        start=False,
                        stop=True,
                    )
                    nc.vector.tensor_add(P_sb[:, fb, :], p_ps[:], bph_sb[:])
                    return first

                # first half of P (covers the previous head's exp wait)
                p_first = emit_p_group(0)
                emit_p_group(1)
                gate_early = p_first.ins

                # --- T1 = X @ Wq_h (uses X symmetry: lhsT = X slices) ---
                T1_ps = big_ps.tile([128, EB, E], F32, tag="big")
                t1_first = None
                for m in range(EB):
                    msl = slice(m * 128, (m + 1) * 128)
                    for ec in range(EB):
                        bi = nc.tensor.matmul(
                            T1_ps[:, m, :],
                            X_sb[:, ec, msl],
                            wq_sb[:, ec, :],
                            start=ec == 0,
                            stop=ec == EB - 1,
                        )
                        t1_first = t1_first or bi
                gate_mid = t1_first.ins
                T1_sb = t1_pool.tile([128, EB, E], BF16, tag="t1")
                for m in range(EB):
                    nc.vector.tensor_copy(T1_sb[:, m, :], T1_ps[:, m, :])

                # second half of P + previous head's s: both independent of
                # T1, they cover the T1 psum->sbuf copy before attT
                emit_p_group(2)
                emit_p_group(3)
                P_tiles.append(P_sb)
                emit_pending_s()

                # --- attT = Wk_h^T @ T1 + uvec (x) bq + bk (x) wvec ---
                attT_ps = big_ps.tile([128, EB, E], F32, tag="big")
                for fb in range(EB):
                    fsl = slice(fb * 128, (fb + 1) * 128)
                    for ec in range(EB):
                        nc.tensor.matmul(
                            attT_ps[:, fb, :],
                            wk_sb[:, ec, fsl],
                            T1_sb[:, ec, :],
                            start=ec == 0,
                            stop=False,
                        )
                    nc.tensor.matmul(
                        attT_ps[:, fb, :],
                        rk1_sb[0:2, u, hl, 0, fsl],
                        rk1_sb[0:2, u, hl, 1, :],
                        start=False,
                        stop=True,
                    )

                # --- exp (softmax numerator, transposed layout) ---
                A_sb = a_pool.tile([128, EB, E], BF16, tag="a")
                for fb in range(EB):
                    nc.scalar.activation(
                        out=A_sb[:, fb, :],
                        in_=attT_ps[:, fb, :],
                        func=mybir.ActivationFunctionType.Exp,
                        scale=SCALE,
                    )
                A_tiles.append(A_sb)
                pending_s = (A_sb, R_tiles)
                gate_hist.append(gate_early)

            # --- U_h = A_h^T @ P_h ; out = U0*r0 + U1*r1 ---
            out_tiles = [
                o_pool.tile([128, E], F32, tag="o", name=f"ot{u}_{i}")
                for i in range(EB)
            ]
            for hl in range(2):
                if hl == 1:
                    emit_pending_s()  # s of this unit's second head
                for eb in range(EB):
                    esl = slice(eb * 128, (eb + 1) * 128)
                    u_tile = u_ps.tile([128, E], F32, tag="u")
                    for fc in range(EB):
                        nc.tensor.matmul(
                            u_tile[:],
                            A_tiles[hl][:, fc, esl],
                            P_tiles[hl][:, fc, :],
                            start=fc == 0,
                            stop=fc == EB - 1,
                        )
                    if hl == 0:
                        nc.vector.tensor_scalar_mul(
                            out_tiles[eb][:], u_tile[:], R_tiles[0][:, eb, 0:1]
                        )
                    else:
                        nc.vector.scalar_tensor_tensor(
                            out_tiles[eb][:],
                            u_tile[:],
                            R_tiles[1][:, eb, 0:1],
                            out_tiles[eb][:],
                            op0=mybir.AluOpType.mult,
                            op1=mybir.AluOpType.add,
                        )
                    if hl == 1:
                        nc.sync.dma_start(
                            out=out_d.ap()[u, eb * 128 : (eb + 1) * 128, :],
                            in_=out_tiles[eb][:],
                        )

    nc.compile()
    return nc


def _get_nc():
    global _CACHED_NC
    if _CACHED_NC is None:
        _CACHED_NC = build_nc()
    return _CACHED_NC


def make_in_maps(x, Wq, bq, Wk, bk, Wv, bv, Wp, bp):
    x = np.asarray(x, np.float32)
    Wq, Wk, Wv, Wp = (np.asarray(a, np.float32) for a in (Wq, Wk, Wv, Wp))
    bq, bk, bv, bp = (np.asarray(a, np.float32) for a in (bq, bk, bv, bp))

    def b16(a):
        return np.ascontiguousarray(a.astype(NP_BF16))

    wp_arr = b16(np.stack([Wp[:N], Wp[N:]]))
    swp = b16(np.stack([Wp[:N].sum(0), Wp[N:].sum(0)])[None])
    bph = np.ascontiguousarray(0.5 * bp)
    in_maps = []
    for c in range(8):
        b = c // 2
        rs = [2 * (c % 2), 2 * (c % 2) + 1]
        heads = [[2 * r + hl for hl in range(2)] for r in rs]
        xs = x[b].sum(0)

        def tile_w(Wm, h):
            # (E, E) -> [p, t, e] with row t*128+p on partition p
            return Wm[:, h::H].reshape(EB, 128, E).transpose(1, 0, 2)

        wq_arr = b16(np.stack([[tile_w(Wq, h) for h in hu] for hu in heads]))
        wk_arr = b16(np.stack([[tile_w(Wk, h) for h in hu] for hu in heads]))
        wv_arr = b16(np.stack([[tile_w(Wv, h) for h in hu] for hu in heads]))
        bqkv = b16(
            np.stack([[[bq[h::H], bk[h::H], bv[h::H]] for h in hu] for hu in heads])[
                :, :, None
            ]
        )
        # rk1[part, u, hl, role, :]: the merged rank-2 bias update of attT,
        #   attT += lhsT.T @ rhs  with  lhsT = rk1[:, u, hl, 0, fsl] (2 x 128)
        #                              rhs  = rk1[:, u, hl, 1, :]   (2 x E)
        # part 0: (Wk^T xs) (x) bq ; part 1: bk (x) (Wq^T xs + N bq)
        rk1 = np.empty((2, 2, 2, 2, E), np.float32)
        for iu, hu in enumerate(heads):
            for ihl, h in enumerate(hu):
                rk1[0, iu, ihl, 0] = Wk[:, h::H].T @ xs
                rk1[0, iu, ihl, 1] = bq[h::H]
                rk1[1, iu, ihl, 0] = bk[h::H]
                rk1[1, iu, ihl, 1] = Wq[:, h::H].T @ xs + np.float32(N) * bq[h::H]
        in_maps.append(
            {
                "xn": b16(x[b]),
                "wq": wq_arr,
                "wk": wk_arr,
                "wv": wv_arr,
                "wp": wp_arr,
                "swp": swp,
                "rk1": b16(rk1),
                "bqkv": bqkv,
                "bph": bph,
                "ones": np.ones((128, 2), NP_BF16),
            }
        )
    return in_maps


def assemble_out(results):
    out = np.empty((B, N, E), np.float32)
    for c in range(8):
        b = c // 2
        for ui in range(2):
            r = 2 * (c % 2) + ui
            out[b, r::4, :] = results[c]["out"][ui]
    return out


def run(inputs, trace=False, **spmd_kwargs):
    """Full pipeline; returns (output, BassKernelResults)."""
    nc = _get_nc()
    in_maps = make_in_maps(**inputs)
    res = run_bass_kernel_spmd(
        nc, in_maps, core_ids=list(range(8)), trace=trace, **spmd_kwargs
    )
    return assemble_out(res.results), res


def kernel(**inputs):
    out, _ = run(inputs)
    return out



# revision 15
# speedup vs baseline: 1.4835x; 1.0856x over previous
"""Trainium2 Bass kernel for the (non-standard) MultiHeadAttention module.

Reference math (B=4, N=2048, E=512, H=8):
    q/k/v  = x @ W{q,k,v} + b          # (B, N, E*H)
    split:   head h takes columns h::H  -> per-head (N, E) matrices
    attT_h = (k_h^T @ q_h) * 1/sqrt(N) # (f, e) -- attention over the E axis
    A_h    = exp(attT_h)               # softmax numerator (logits reach ~33,
                                       #  exp ~1e14: fits f32/bf16, NOT fp8)
    s_h[e] = sum_f A_h[f, e]
    out row n' = 4e + r gets  sum_hl (A_h^T/s_h) @ P_h + bp
      for h = 2r + hl  (consequence of the reference's raw
      (B,E,H,N)->(B,N,E*H) reshape before the output projection), where
    P_h    = v_h^T @ Wp_half(hl) + bp/2

Key algebraic refactors (this module attends over the E axis and contracts
over n, so everything collapses into E x E space):
  * Gram matrix  X = x_b^T @ x_b  (E x E, once per core):
      attT_h = Wk_h^T X Wq_h + (Wk_h^T xs) (x) bq_h
               + bk_h (x) (Wq_h^T xs + N bq_h),   xs = colsum(x_b)
    -- eliminates the q/k projections entirely.  The two rank-1 updates are
    merged into one contraction-2 matmul (rk1 tensor).
  * (A @ v^T) @ Wp == A @ (v^T @ Wp) and
    v_h^T @ Wp_hl == Wv_h^T @ G_hl + bv_h (x) colsum(Wp_hl)  with
    G_hl = x_b^T @ Wp_hl computed once per core -- eliminates the v
    projection and the big P matmuls.  bv (x) swp and the bp/2 constant are
    one contraction-2 matmul (pb2 tensor).
  * softmax normalization at the very end:
    out = U0*r0 + U1*r1,  U_h = A_h^T @ P_h,  r_h = 1/s_h.

Everything runs in bf16 with f32 PSUM accumulation: fp32/f32r moving
operands stream at half rate on real TRN2 HW, and fp8 fails BOTH ways --
DoubleRow matmuls measure ~600ns (slower than 2 bf16 matmuls) and, with
sharply peaked attention (logits to +-33), out ~= P[argmax f, :], so fp8's
~3-5% error on the P chain lands directly on the output (measured 5.2e-2
vs the 2e-2 budget; bf16 measures ~5e-3).

Sharding: 16 independent units (b, r), b in 0..3, r in 0..3; unit (b, r)
owns heads {2r, 2r+1} and produces output rows out[b, r::4, :].  Two units
per core, batch-major:  core c -> b = c//2, r in {2*(c%2), 2*(c%2)+1}.
No inter-core communication.
"""

import numpy as np
import ml_dtypes
from contextlib import ExitStack

import concourse.bass as bass
import concourse.mybir as mybir
import concourse.tile as tile
from concourse import bacc
from concourse.bass_utils import run_bass_kernel_spmd

B, N, E, H = 4, 2048, 512, 8
NT = N // 128          # 16 contraction chunks of 128 over n
NP2 = NT // 2          # 8 chunk-pairs (xn tiles hold 2 chunks)
EB = E // 128          # 4 blocks of 128 over e/f
SCALE = float(1.0 / np.sqrt(np.float32(N)))
F32 = mybir.dt.float32
BF16 = mybir.dt.bfloat16
NP_BF16 = ml_dtypes.bfloat16
PSUM = bass.MemorySpace.PSUM
EXP = mybir.ActivationFunctionType.Exp
COPY = mybir.ActivationFunctionType.Copy

_CACHED_NC = None


def _rows_ap(dram, row0, nrow, width, nbatch):
    """AP over dram rows [row0, row0+nrow*nbatch) shaped [nrow, nbatch, width]
    (partition-first chunked layout for a [128, nbatch, width] SBUF tile)."""
    return bass.AP(
        tensor=dram,
        offset=row0 * width,
        ap=[[width, nrow], [nrow * width, nbatch], [1, width]],
    )


def build_nc():
    nc = bacc.Bacc("TRN2", target_bir_lowering=False, debug=False)

    xn_d = nc.dram_tensor("xn", (N, E), BF16, kind="ExternalInput")
    wp_d = nc.dram_tensor("wp", (2, N, E), BF16, kind="ExternalInput")
    wq_d = nc.dram_tensor("wq", (2, 2, 128, EB, E), BF16, kind="ExternalInput")
    wk_d = nc.dram_tensor("wk", (2, 2, 128, EB, E), BF16, kind="ExternalInput")
    wv_d = nc.dram_tensor("wv", (2, 2, 128, EB, E), BF16, kind="ExternalInput")
    pb2_d = nc.dram_tensor("pb2", (2, 2, 2, 2, E), BF16, kind="ExternalInput")
    rk1_d = nc.dram_tensor("rk1", (2, 2, 2, 2, E), BF16, kind="ExternalInput")
    out_d = nc.dram_tensor("out", (2, E, E), F32, kind="ExternalOutput")

    with tile.TileContext(nc) as tc, ExitStack() as ctx:
        consts = ctx.enter_context(tc.tile_pool(name="consts", bufs=1))
        stream = ctx.enter_context(tc.tile_pool(name="stream", bufs=8))
        wqkv_pool = ctx.enter_context(tc.tile_pool(name="wqkv", bufs=2))
        t1_pool = ctx.enter_context(tc.tile_pool(name="t1", bufs=1))
        a_pool = ctx.enter_context(tc.tile_pool(name="a", bufs=2))
        p_pool = ctx.enter_context(tc.tile_pool(name="p", bufs=1))
        o_pool = ctx.enter_context(tc.tile_pool(name="o", bufs=4))
        r_pool = ctx.enter_context(tc.tile_pool(name="r", bufs=4))
        mm_ps = ctx.enter_context(tc.tile_pool(name="mmps", bufs=2, space=PSUM))
        big_ps = ctx.enter_context(tc.tile_pool(name="bigps", bufs=1, space=PSUM))
        u_ps = ctx.enter_context(tc.tile_pool(name="ups", bufs=2, space=PSUM))

        # ---- PE warm-up: dummy matmuls on a memset tile so the HAM clock
        # gate flips to 8/8 before the real work arrives ----
        warm_sb = consts.tile([128, E], BF16, tag="warm")
        nc.vector.memset(warm_sb[:], 0.0)
        warm_ps = mm_ps.tile([128, E], F32, tag="mm", name="warm")
        for i in range(8):
            nc.tensor.matmul(
                warm_ps[:], warm_sb[:, 0:128], warm_sb[:], start=i == 0, stop=i == 7
            )

        # ---- streamed inputs, batched DMAs ----
        # sync: xn (8 x 2-chunk); gpsimd: wp halves (8 x 4-chunk) + per-head
        # wv/wq; scalar: consts + per-head wk.
        xn_sb = []
        for i in range(8):
            t = consts.tile([128, 2, E], BF16, tag=f"xn{i}", name=f"xn{i}")
            nc.sync.dma_start(out=t[:], in_=_rows_ap(xn_d, i * 256, 128, E, 2))
            xn_sb.append(t)
        wp_sb = {}
        for hl in range(2):
            for i in range(4):
                t = stream.tile([128, 4, E], BF16, tag="wp", name=f"wp_{hl}_{i}")
                nc.gpsimd.dma_start(
                    out=t[:], in_=_rows_ap(wp_d, hl * N + i * 512, 128, E, 4)
                )
                wp_sb[(hl, i)] = t

        # ---- other resident constants (scalar queue) ----
        pb2_sb = consts.tile([2, 2, 2, 2, E], BF16, tag="pb2")
        nc.scalar.dma_start(out=pb2_sb[:], in_=pb2_d.ap())
        rk1_sb = consts.tile([2, 2, 2, 2, E], BF16, tag="rk1")
        nc.scalar.dma_start(out=rk1_sb[:], in_=rk1_d.ap())

        # ---- P tiles (persistent) + all-ones vector for s ----
        P_sbs = [
            p_pool.tile([128, EB, E], BF16, tag=f"p{i}", name=f"P{i}")
            for i in range(2)
        ]
        ones_sb = consts.tile([128, 2], BF16, tag="ones")
        nc.vector.memset(ones_sb[:], 1.0)

        # ---- pass 1: X = x^T x (big arena) + G0 = x^T Wp0 (mm/u slots) ----
        X_ps = big_ps.tile([128, EB, E], F32, tag="big")
        g_sb = [
            consts.tile([128, EB, E], BF16, tag=f"g{hl}", name=f"g{hl}")
            for hl in range(2)
        ]
        g0_slots = [
            mm_ps.tile([128, E], F32, tag="mm", name="g0a"),
            mm_ps.tile([128, E], F32, tag="mm", name="g0b"),
            u_ps.tile([128, E], F32, tag="u", name="g0c"),
            u_ps.tile([128, E], F32, tag="u", name="g0d"),
        ]
        gate_g0 = None
        for np_ in range(NP2):
            xt = xn_sb[np_]
            wpt = wp_sb[(0, np_ // 2)]
            jw = 2 * (np_ % 2)
            for m in range(EB):
                msl = slice(m * 128, (m + 1) * 128)
                for j in range(2):
                    nc.tensor.matmul(
                        X_ps[:, m, :],
                        xt[:, j, msl],
                        xt[:, j, :],
                        start=np_ == 0 and j == 0,
                        stop=np_ == NP2 - 1 and j == 1,
                    )
                    g_bi = nc.tensor.matmul(
                        g0_slots[m][:],
                        xt[:, j, msl],
                        wpt[:, jw + j, :],
                        start=np_ == 0 and j == 0,
                        stop=np_ == NP2 - 1 and j == 1,
                    )
                if np_ == 1 and m == 0:
                    gate_g0 = g_bi.ins
        # g casts first (they gate pass-2 psum reuse), X casts after
        nc.vector.tensor_copy(g_sb[0][:, 0, :], g0_slots[0][:])
        nc.scalar.activation(g_sb[0][:, 1, :], g0_slots[1][:], COPY)
        nc.vector.tensor_copy(g_sb[0][:, 2, :], g0_slots[2][:])
        nc.scalar.activation(g_sb[0][:, 3, :], g0_slots[3][:], COPY)
        X_sb = consts.tile([128, EB, E], BF16, tag="X")
        nc.vector.tensor_copy(X_sb[:, 0, :], X_ps[:, 0, :])
        nc.scalar.activation(X_sb[:, 1, :], X_ps[:, 1, :], COPY)
        nc.vector.tensor_copy(X_sb[:, 2, :], X_ps[:, 2, :])
        nc.scalar.activation(X_sb[:, 3, :], X_ps[:, 3, :], COPY)

        # ---- pass 2: G1 = x^T Wp1 ----
        g1_slots = [
            mm_ps.tile([128, E], F32, tag="mm", name="g1a"),
            mm_ps.tile([128, E], F32, tag="mm", name="g1b"),
            u_ps.tile([128, E], F32, tag="u", name="g1c"),
            u_ps.tile([128, E], F32, tag="u", name="g1d"),
        ]
        gate_g1 = None
        for np_ in range(NP2):
            xt = xn_sb[np_]
            wpt = wp_sb[(1, np_ // 2)]
            jw = 2 * (np_ % 2)
            for m in range(EB):
                msl = slice(m * 128, (m + 1) * 128)
                for j in range(2):
                    g_bi = nc.tensor.matmul(
                        g1_slots[m][:],
                        xt[:, j, msl],
                        wpt[:, jw + j, :],
                        start=np_ == 0 and j == 0,
                        stop=np_ == NP2 - 1 and j == 1,
                    )
                if np_ == 1 and m == 0:
                    gate_g1 = g_bi.ins
        nc.vector.tensor_copy(g_sb[1][:, 0, :], g1_slots[0][:])
        nc.scalar.activation(g_sb[1][:, 1, :], g1_slots[1][:], COPY)
        nc.vector.tensor_copy(g_sb[1][:, 2, :], g1_slots[2][:])
        nc.scalar.activation(g_sb[1][:, 3, :], g1_slots[3][:], COPY)

        gate_hist = [gate_g0, gate_g1]  # per-head early gates
        pending_s = None

        def emit_pending_s():
            nonlocal pending_s
            if pending_s is None:
                return
            A_sb, R_list = pending_s
            pending_s = None
            s_ps = mm_ps.tile([128, EB, 2], F32, tag="mm")
            for eb in range(EB):
                esl = slice(eb * 128, (eb + 1) * 128)
                for fc in range(EB):
                    nc.tensor.matmul(
                        s_ps[:, eb, :],
                        A_sb[:, fc, esl],
                        ones_sb[:],
                        start=fc == 0,
                        stop=fc == EB - 1,
                    )
            r_sb = r_pool.tile([128, EB, 2], F32, tag="r")
            nc.vector.reciprocal(out=r_sb[:], in_=s_ps[:])
            R_list.append(r_sb)

        for u in range(2):
            A_tiles, R_tiles = [], []
            for hl in range(2):
                # --- weights for head (u, hl), prefetch-gated ---
                wv_sb = wqkv_pool.tile([128, EB, E], BF16, tag="wv")
                wv_bi = nc.gpsimd.dma_start(out=wv_sb[:], in_=wv_d.ap()[u, hl])
                wq_sb = wqkv_pool.tile([128, EB, E], BF16, tag="wq")
                wq_bi = nc.gpsimd.dma_start(out=wq_sb[:], in_=wq_d.ap()[u, hl])
                wk_sb = wqkv_pool.tile([128, EB, E], BF16, tag="wk")
                wk_bi = nc.scalar.dma_start(out=wk_sb[:], in_=wk_d.ap()[u, hl])
                gate = gate_hist[-2]  # two head-phases back
                for bi in (wv_bi, wq_bi, wk_bi):
                    tile.add_dep_helper(bi.ins, gate, reason="delay prefetch")

                P_sb = P_sbs[hl]

                # --- P_h = Wv_h^T @ G_hl + bv (x) swp + bp/2 ---
                def emit_p_group(fb, use_act):
                    fsl = slice(fb * 128, (fb + 1) * 128)
                    p_ps = (u_ps if fb % 2 == 0 else mm_ps).tile(
                        [128, E], F32, tag="u" if fb % 2 == 0 else "mm",
                        name=f"pp{fb}",
                    )
                    first = None
                    for ec in range(EB):
                        bi = nc.tensor.matmul(
                            p_ps[:],
                            wv_sb[:, ec, fsl],
                            g_sb[hl][:, ec, :],
                            start=ec == 0,
                            stop=False,
                        )
                        first = first or bi
                    nc.tensor.matmul(
                        p_ps[:],
                        pb2_sb[0:2, u, hl, 0, fsl],
                        pb2_sb[0:2, u, hl, 1, :],
                        start=False,
                        stop=True,
                    )
                    if use_act:
                        nc.scalar.activation(P_sb[:, fb, :], p_ps[:], COPY)
                    else:
                        nc.vector.tensor_copy(P_sb[:, fb, :], p_ps[:])
                    return first

                def emit_t1():
                    T1_ps = big_ps.tile([128, EB, E], F32, tag="big")
                    for m in range(EB):
                        msl = slice(m * 128, (m + 1) * 128)
                        for ec in range(EB):
                            nc.tensor.matmul(
                                T1_ps[:, m, :],
                                X_sb[:, ec, msl],
                                wq_sb[:, ec, :],
                                start=ec == 0,
                                stop=ec == EB - 1,
                            )
                    T1_sb = t1_pool.tile([128, EB, E], BF16, tag="t1")
                    nc.vector.tensor_copy(T1_sb[:, 0, :], T1_ps[:, 0, :])
                    nc.scalar.activation(T1_sb[:, 1, :], T1_ps[:, 1, :], COPY)
                    nc.vector.tensor_copy(T1_sb[:, 2, :], T1_ps[:, 2, :])
                    nc.scalar.activation(T1_sb[:, 3, :], T1_ps[:, 3, :], COPY)
                    return T1_sb

                if u == 0 and hl == 0:
                    # first head: nothing to cover before T1, and P's psum
                    # slots want the G1 casts done -- T1's matmuls cover that
                    T1_sb = emit_t1()
                    p_first = emit_p_group(0, False)
                    emit_p_group(1, True)
                    emit_p_group(2, True)
                    emit_p_group(3, False)
                else:
                    # first half of P covers the previous head's exp wait
                    p_first = emit_p_group(0, False)
                    emit_p_group(1, True)
                    T1_sb = emit_t1()
                    # second half of P covers the T1 casts
                    emit_p_group(2, True)
                    emit_p_group(3, False)
                gate_hist.append(p_first.ins)
                emit_pending_s()

                # --- attT = Wk_h^T @ T1 + rk1 rank-2 update ---
                attT_ps = big_ps.tile([128, EB, E], F32, tag="big")
                for fb in range(EB):
                    fsl = slice(fb * 128, (fb + 1) * 128)
                    for ec in range(EB):
                        nc.tensor.matmul(
                            attT_ps[:, fb, :],
                            wk_sb[:, ec, fsl],
                            T1_sb[:, ec, :],
                            start=ec == 0,
                            stop=False,
                        )
                    nc.tensor.matmul(
                        attT_ps[:, fb, :],
                        rk1_sb[0:2, u, hl, 0, fsl],
                        rk1_sb[0:2, u, hl, 1, :],
                        start=False,
                        stop=True,
                    )

                # --- exp (softmax numerator, transposed layout) ---
                A_sb = a_pool.tile([128, EB, E], BF16, tag="a")
                for fb in range(EB):
                    nc.scalar.activation(
                        out=A_sb[:, fb, :],
                        in_=attT_ps[:, fb, :],
                        func=EXP,
                        scale=SCALE,
                    )
                A_tiles.append(A_sb)
                pending_s = (A_sb, R_tiles)

            # --- U_h = A_h^T @ P_h ; out = U0*r0 + U1*r1 ---
            out_tiles = [
                o_pool.tile([128, E], F32, tag="o", name=f"ot{u}_{i}")
                for i in range(EB)
            ]
            for hl in range(2):
                if hl == 1:
                    emit_pending_s()  # s of this unit's second head
                for eb in range(EB):
                    esl = slice(eb * 128, (eb + 1) * 128)
                    u_tile = (u_ps if eb % 2 == 0 else mm_ps).tile(
                        [128, E], F32, tag="u" if eb % 2 == 0 else "mm",
                        name=f"ut{hl}_{eb}",
                    )
                    for fc in range(EB):
                        nc.tensor.matmul(
                            u_tile[:],
                            A_tiles[hl][:, fc, esl],
                            P_sbs[hl][:, fc, :],
                            start=fc == 0,
                            stop=fc == EB - 1,
                        )
                    if hl == 0:
                        nc.vector.tensor_scalar_mul(
                            out_tiles[eb][:], u_tile[:], R_tiles[0][:, eb, 0:1]
                        )
                    else:
                        nc.vector.scalar_tensor_tensor(
                            out_tiles[eb][:],
                            u_tile[:],
                            R_tiles[1][:, eb, 0:1],
                            out_tiles[eb][:],
                            op0=mybir.AluOpType.mult,
                            op1=mybir.AluOpType.add,
                        )
                        nc.sync.dma_start(
                            out=out_d.ap()[u, eb * 128 : (eb + 1) * 128, :],
                            in_=out_tiles[eb][:],
                        )

    nc.compile()
    return nc


def _get_nc():
    global _CACHED_NC
    if _CACHED_NC is None:
        _CACHED_NC = build_nc()
    return _CACHED_NC


def make_in_maps(x, Wq, bq, Wk, bk, Wv, bv, Wp, bp):
    x = np.asarray(x, np.float32)
    Wq, Wk, Wv, Wp = (np.asarray(a, np.float32) for a in (Wq, Wk, Wv, Wp))
    bq, bk, bv, bp = (np.asarray(a, np.float32) for a in (bq, bk, bv, bp))

    def b16(a):
        return np.ascontiguousarray(a.astype(NP_BF16))

    wp_arr = b16(np.stack([Wp[:N], Wp[N:]]))
    swp = np.stack([Wp[:N].sum(0), Wp[N:].sum(0)])  # (2, E)
    in_maps = []
    for c in range(8):
        b = c // 2
        rs = [2 * (c % 2), 2 * (c % 2) + 1]
        heads = [[2 * r + hl for hl in range(2)] for r in rs]
        xs = x[b].sum(0)

        def tile_w(Wm, h):
            # (E, E) -> [p, t, e] with row t*128+p on partition p
            return Wm[:, h::H].reshape(EB, 128, E).transpose(1, 0, 2)

        wq_arr = b16(np.stack([[tile_w(Wq, h) for h in hu] for hu in heads]))
        wk_arr = b16(np.stack([[tile_w(Wk, h) for h in hu] for hu in heads]))
        wv_arr = b16(np.stack([[tile_w(Wv, h) for h in hu] for hu in heads]))
        # pb2[part, u, hl, role, :]: rank-2 P bias update:
        #   part 0: bv (lhsT) / swp_hl (rhs);  part 1: ones (lhsT) / bp/2
        pb2 = np.empty((2, 2, 2, 2, E), np.float32)
        # rk1[part, u, hl, role, :]: merged rank-2 bias update of attT:
        #   part 0: (Wk^T xs) (x) bq ; part 1: bk (x) (Wq^T xs + N bq)
        rk1 = np.empty((2, 2, 2, 2, E), np.float32)
        for iu, hu in enumerate(heads):
            for ihl, h in enumerate(hu):
                pb2[0, iu, ihl, 0] = bv[h::H]
                pb2[0, iu, ihl, 1] = swp[ihl]
                pb2[1, iu, ihl, 0] = 1.0
                pb2[1, iu, ihl, 1] = 0.5 * bp
                rk1[0, iu, ihl, 0] = Wk[:, h::H].T @ xs
                rk1[0, iu, ihl, 1] = bq[h::H]
                rk1[1, iu, ihl, 0] = bk[h::H]
                rk1[1, iu, ihl, 1] = Wq[:, h::H].T @ xs + np.float32(N) * bq[h::H]
        in_maps.append(
            {
                "xn": b16(x[b]),
                "wp": wp_arr,
                "wq": wq_arr,
                "wk": wk_arr,
                "wv": wv_arr,
                "pb2": b16(pb2),
                "rk1": b16(rk1),
            }
        )
    return in_maps


def assemble_out(results):
    out = np.empty((B, N, E), np.float32)
    for c in range(8):
        b = c // 2
        for ui in range(2):
            r = 2 * (c % 2) + ui
            out[b, r::4, :] = results[c]["out"][ui]
    return out


def run(inputs, trace=False, **spmd_kwargs):
    """Full pipeline; returns (output, BassKernelResults)."""
    nc = _get_nc()
    in_maps = make_in_maps(**inputs)
    res = run_bass_kernel_spmd(
        nc, in_maps, core_ids=list(range(8)), trace=trace, **spmd_kwargs
    )
    return assemble_out(res.results), res


def kernel(**inputs):
    out, _ = run(inputs)
    return out


# revision 17
# speedup vs baseline: 1.5622x; 1.0531x over previous
"""Trainium2 Bass kernel for the (non-standard) MultiHeadAttention module.

Reference math (B=4, N=2048, E=512, H=8):
    q/k/v  = x @ W{q,k,v} + b          # (B, N, E*H)
    split:   head h takes columns h::H  -> per-head (N, E) matrices
    attT_h = (k_h^T @ q_h) * 1/sqrt(N) # (f, e) -- attention over the E axis
    A_h    = exp(attT_h)               # softmax numerator (logits reach ~33,
                                       #  exp ~1e14: fits f32/bf16, NOT fp8)
    s_h[e] = sum_f A_h[f, e]
    out row n' = 4e + r gets  sum_hl (A_h^T/s_h) @ P_h + bp
      for h = 2r + hl  (consequence of the reference's raw
      (B,E,H,N)->(B,N,E*H) reshape before the output projection), where
    P_h    = v_h^T @ Wp_half(hl) + bp/2

Key algebraic refactors (this module attends over the E axis and contracts
over n, so everything collapses into E x E space):
  * Gram matrix  X = x_b^T @ x_b  (E x E, once per core):
      attT_h = Wk_h^T X Wq_h + (Wk_h^T xs) (x) bq_h
               + bk_h (x) (Wq_h^T xs + N bq_h),   xs = colsum(x_b)
    -- eliminates the q/k projections entirely.  The two rank-1 updates are
    merged into one contraction-2 matmul (rk1 tensor).
  * (A @ v^T) @ Wp == A @ (v^T @ Wp) and
    v_h^T @ Wp_hl == Wv_h^T @ G_hl + bv_h (x) colsum(Wp_hl)  with
    G_hl = x_b^T @ Wp_hl computed once per core -- eliminates the v
    projection and the big P matmuls.  bv (x) swp and the bp/2 constant are
    one contraction-2 matmul (pb2 tensor).
  * softmax normalization at the very end:
    out = U0*r0 + U1*r1,  U_h = A_h^T @ P_h,  r_h = 1/s_h.

Everything runs in bf16 with f32 PSUM accumulation: fp32/f32r moving
operands stream at half rate on real TRN2 HW, and fp8 fails BOTH ways --
DoubleRow matmuls measure ~600ns (slower than 2 bf16 matmuls) and, with
sharply peaked attention (logits to +-33), out ~= P[argmax f, :], so fp8's
~3-5% error on the P chain lands directly on the output (measured 5.2e-2
vs the 2e-2 budget; bf16 measures ~5e-3).

Sharding: 16 independent units (b, r), b in 0..3, r in 0..3; unit (b, r)
owns heads {2r, 2r+1} and produces output rows out[b, r::4, :].  Two units
per core, batch-major:  core c -> b = c//2, r in {2*(c%2), 2*(c%2)+1}.
No inter-core communication.
"""

import numpy as np
import ml_dtypes
from contextlib import ExitStack

import concourse.bass as bass
import concourse.mybir as mybir
import concourse.tile as tile
from concourse import bacc
from concourse.bass_utils import run_bass_kernel_spmd

B, N, E, H = 4, 2048, 512, 8
NT = N // 128          # 16 contraction chunks of 128 over n
NP2 = NT // 2          # 8 chunk-pairs (xn tiles hold 2 chunks)
EB = E // 128          # 4 blocks of 128 over e/f
SCALE = float(1.0 / np.sqrt(np.float32(N)))
F32 = mybir.dt.float32
BF16 = mybir.dt.bfloat16
NP_BF16 = ml_dtypes.bfloat16
PSUM = bass.MemorySpace.PSUM
EXP = mybir.ActivationFunctionType.Exp
COPY = mybir.ActivationFunctionType.Copy

_CACHED_NC = None


def _rows_ap(dram, row0, nrow, width, nbatch):
    """AP over dram rows [row0, row0+nrow*nbatch) shaped [nrow, nbatch, width]
    (partition-first chunked layout for a [128, nbatch, width] SBUF tile)."""
    return bass.AP(
        tensor=dram,
        offset=row0 * width,
        ap=[[width, nrow], [nrow * width, nbatch], [1, width]],
    )


def build_nc():
    nc = bacc.Bacc("TRN2", target_bir_lowering=False, debug=False)

    xn_d = nc.dram_tensor("xn", (N, E), BF16, kind="ExternalInput")
    wp_d = nc.dram_tensor("wp", (2, N, E), BF16, kind="ExternalInput")
    wq_d = nc.dram_tensor("wq", (2, 2, 128, EB, E), BF16, kind="ExternalInput")
    wk_d = nc.dram_tensor("wk", (2, 2, 128, EB, E), BF16, kind="ExternalInput")
    wv_d = nc.dram_tensor("wv", (2, 2, 128, EB, E), BF16, kind="ExternalInput")
    pb2_d = nc.dram_tensor("pb2", (2, 2, 2, 2, E), BF16, kind="ExternalInput")
    rk1_d = nc.dram_tensor("rk1", (2, 2, 2, 2, E), BF16, kind="ExternalInput")
    out_d = nc.dram_tensor("out", (2, E, E), F32, kind="ExternalOutput")

    with tile.TileContext(nc) as tc, ExitStack() as ctx:
        consts = ctx.enter_context(tc.tile_pool(name="consts", bufs=1))
        stream = ctx.enter_context(tc.tile_pool(name="stream", bufs=8))
        wqkv_pool = ctx.enter_context(tc.tile_pool(name="wqkv", bufs=2))
        t1_pool = ctx.enter_context(tc.tile_pool(name="t1", bufs=1))
        a_pool = ctx.enter_context(tc.tile_pool(name="a", bufs=2))
        p_pool = ctx.enter_context(tc.tile_pool(name="p", bufs=1))
        o_pool = ctx.enter_context(tc.tile_pool(name="o", bufs=4))
        r_pool = ctx.enter_context(tc.tile_pool(name="r", bufs=4))
        mm_ps = ctx.enter_context(tc.tile_pool(name="mmps", bufs=2, space=PSUM))
        big_ps = ctx.enter_context(tc.tile_pool(name="bigps", bufs=1, space=PSUM))
        u_ps = ctx.enter_context(tc.tile_pool(name="ups", bufs=2, space=PSUM))

        # ---- PE warm-up: dummy matmuls on a memset tile so the HAM clock
        # gate flips to 8/8 before the real work arrives ----
        warm_sb = consts.tile([128, E], BF16, tag="warm")
        nc.vector.memset(warm_sb[:], 0.0)
        warm_ps = mm_ps.tile([128, E], F32, tag="mm", name="warm")
        for i in range(8):
            nc.tensor.matmul(
                warm_ps[:], warm_sb[:, 0:128], warm_sb[:], start=i == 0, stop=i == 7
            )

        # ---- streamed inputs, batched DMAs ----
        # sync: xn (8 x 2-chunk); gpsimd: wp halves (8 x 4-chunk) + per-head
        # wv/wq; scalar: consts + per-head wk.
        xn_sb = []
        for i in range(8):
            t = consts.tile([128, 2, E], BF16, tag=f"xn{i}", name=f"xn{i}")
            nc.sync.dma_start(out=t[:], in_=_rows_ap(xn_d, i * 256, 128, E, 2))
            xn_sb.append(t)
        wp_sb = {}
        wp1_bis = []
        for hl in range(2):
            for i in range(8):
                t = stream.tile([128, 2, E], BF16, tag="wp", name=f"wp_{hl}_{i}", bufs=16)
                bi = nc.gpsimd.dma_start(
                    out=t[:], in_=_rows_ap(wp_d, hl * N + i * 256, 128, E, 2)
                )
                if hl == 1:
                    wp1_bis.append(bi)
                wp_sb[(hl, i)] = t

        # ---- other resident constants (scalar queue) ----
        pb2_sb = consts.tile([2, 2, 2, 2, E], BF16, tag="pb2")
        nc.scalar.dma_start(out=pb2_sb[:], in_=pb2_d.ap())
        rk1_sb = consts.tile([2, 2, 2, 2, E], BF16, tag="rk1")
        nc.scalar.dma_start(out=rk1_sb[:], in_=rk1_d.ap())

        # ---- P tiles (persistent) + all-ones vector for s ----
        P_sbs = [
            p_pool.tile([128, EB, E], BF16, tag=f"p{i}", name=f"P{i}")
            for i in range(2)
        ]
        ones_sb = consts.tile([128, 2], BF16, tag="ones")
        nc.vector.memset(ones_sb[:], 1.0)

        # ---- pass 1: X = x^T x (big arena) + G0 = x^T Wp0 (mm/u slots) ----
        X_ps = big_ps.tile([128, EB, E], F32, tag="big")
        g_sb = [
            consts.tile([128, EB, E], BF16, tag=f"g{hl}", name=f"g{hl}")
            for hl in range(2)
        ]
        g0_slots = [
            mm_ps.tile([128, E], F32, tag="mm", name="g0a"),
            mm_ps.tile([128, E], F32, tag="mm", name="g0b"),
            u_ps.tile([128, E], F32, tag="u", name="g0c"),
            u_ps.tile([128, E], F32, tag="u", name="g0d"),
        ]
        gate_g0 = None
        for np_ in range(NP2):
            xt = xn_sb[np_]
            wpt = wp_sb[(0, np_)]
            # X first (triangle: only columns >= m), then G0 -- gives the wp
            # stream extra slack so its DMA never stalls the PE
            for m in range(EB):
                msl = slice(m * 128, (m + 1) * 128)
                for j in range(2):
                    nc.tensor.matmul(
                        X_ps[:, m, m * 128 :],
                        xt[:, j, msl],
                        xt[:, j, m * 128 :],
                        start=np_ == 0 and j == 0,
                        stop=np_ == NP2 - 1 and j == 1,
                    )
            for m in range(EB):
                msl = slice(m * 128, (m + 1) * 128)
                for j in range(2):
                    g_bi = nc.tensor.matmul(
                        g0_slots[m][:],
                        xt[:, j, msl],
                        wpt[:, j, :],
                        start=np_ == 0 and j == 0,
                        stop=np_ == NP2 - 1 and j == 1,
                    )
                if np_ == 1 and m == 0:
                    gate_g0 = g_bi.ins
            if np_ == 1:
                for bi in wp1_bis:
                    tile.add_dep_helper(bi.ins, gate_g0, reason="delay wp1")
        # g casts first (they gate pass-2 psum reuse), X casts after
        nc.vector.tensor_copy(g_sb[0][:, 0, :], g0_slots[0][:])
        nc.scalar.activation(g_sb[0][:, 1, :], g0_slots[1][:], COPY)
        nc.vector.tensor_copy(g_sb[0][:, 2, :], g0_slots[2][:])
        nc.scalar.activation(g_sb[0][:, 3, :], g0_slots[3][:], COPY)
        X_sb = consts.tile([128, EB, E], BF16, tag="X")
        nc.vector.tensor_copy(X_sb[:, 0, :], X_ps[:, 0, :])
        nc.scalar.activation(X_sb[:, 1, 128:], X_ps[:, 1, 128:], COPY)
        nc.vector.tensor_copy(X_sb[:, 2, 256:], X_ps[:, 2, 256:])
        nc.scalar.activation(X_sb[:, 3, 384:], X_ps[:, 3, 384:], COPY)

        # ---- pass 2: G1 = x^T Wp1 ----
        g1_slots = [
            mm_ps.tile([128, E], F32, tag="mm", name="g1a"),
            mm_ps.tile([128, E], F32, tag="mm", name="g1b"),
            u_ps.tile([128, E], F32, tag="u", name="g1c"),
            u_ps.tile([128, E], F32, tag="u", name="g1d"),
        ]
        gate_g1 = None
        for np_ in range(NP2):
            xt = xn_sb[np_]
            wpt = wp_sb[(1, np_)]
            for m in range(EB):
                msl = slice(m * 128, (m + 1) * 128)
                for j in range(2):
                    g_bi = nc.tensor.matmul(
                        g1_slots[m][:],
                        xt[:, j, msl],
                        wpt[:, j, :],
                        start=np_ == 0 and j == 0,
                        stop=np_ == NP2 - 1 and j == 1,
                    )
                if np_ == 1 and m == 0:
                    gate_g1 = g_bi.ins
            if np_ == 2:
                # X is symmetric: fill the lower-triangle blocks with
                # SBUF->SBUF DMA transposes of the upper blocks (no PE cost)
                engs = [nc.sync, nc.scalar]
                k = 0
                for mt in range(1, EB):
                    for ct in range(mt):
                        engs[k % 2].dma_start_transpose(
                            out=X_sb[:, mt, ct * 128 : (ct + 1) * 128],
                            in_=X_sb[:, ct, mt * 128 : (mt + 1) * 128],
                        )
                        k += 1
        nc.vector.tensor_copy(g_sb[1][:, 0, :], g1_slots[0][:])
        nc.scalar.activation(g_sb[1][:, 1, :], g1_slots[1][:], COPY)
        nc.vector.tensor_copy(g_sb[1][:, 2, :], g1_slots[2][:])
        nc.scalar.activation(g_sb[1][:, 3, :], g1_slots[3][:], COPY)

        gate_hist = [gate_g0, gate_g1]  # per-head early gates
        pending_s = None

        def emit_pending_s():
            nonlocal pending_s
            if pending_s is None:
                return
            A_sb, R_list = pending_s
            pending_s = None
            s_ps = mm_ps.tile([128, EB, 2], F32, tag="mm")
            for eb in range(EB):
                esl = slice(eb * 128, (eb + 1) * 128)
                for fc in range(EB):
                    nc.tensor.matmul(
                        s_ps[:, eb, :],
                        A_sb[:, fc, esl],
                        ones_sb[:],
                        start=fc == 0,
                        stop=fc == EB - 1,
                    )
            r_sb = r_pool.tile([128, EB, 2], F32, tag="r")
            nc.vector.reciprocal(out=r_sb[:], in_=s_ps[:])
            R_list.append(r_sb)

        for u in range(2):
            A_tiles, R_tiles = [], []
            for hl in range(2):
                # --- weights for head (u, hl), prefetch-gated ---
                wv_sb = wqkv_pool.tile([128, EB, E], BF16, tag="wv")
                wv_bi = nc.gpsimd.dma_start(out=wv_sb[:], in_=wv_d.ap()[u, hl])
                wq_sb = wqkv_pool.tile([128, EB, E], BF16, tag="wq")
                wq_bi = nc.gpsimd.dma_start(out=wq_sb[:], in_=wq_d.ap()[u, hl])
                wk_sb = wqkv_pool.tile([128, EB, E], BF16, tag="wk")
                wk_bi = nc.scalar.dma_start(out=wk_sb[:], in_=wk_d.ap()[u, hl])
                gate = gate_hist[-2]  # two head-phases back
                for bi in (wv_bi, wq_bi, wk_bi):
                    tile.add_dep_helper(bi.ins, gate, reason="delay prefetch")

                P_sb = P_sbs[hl]

                # --- P_h = Wv_h^T @ G_hl + bv (x) swp + bp/2 ---
                def emit_p_group(fb, use_act):
                    fsl = slice(fb * 128, (fb + 1) * 128)
                    p_ps = (u_ps if fb % 2 == 0 else mm_ps).tile(
                        [128, E], F32, tag="u" if fb % 2 == 0 else "mm",
                        name=f"pp{fb}",
                    )
                    first = None
                    for ec in range(EB):
                        bi = nc.tensor.matmul(
                            p_ps[:],
                            wv_sb[:, ec, fsl],
                            g_sb[hl][:, ec, :],
                            start=ec == 0,
                            stop=False,
                        )
                        first = first or bi
                    nc.tensor.matmul(
                        p_ps[:],
                        pb2_sb[0:2, u, hl, 0, fsl],
                        pb2_sb[0:2, u, hl, 1, :],
                        start=False,
                        stop=True,
                    )
                    if use_act:
                        nc.scalar.activation(P_sb[:, fb, :], p_ps[:], COPY)
                    else:
                        nc.vector.tensor_copy(P_sb[:, fb, :], p_ps[:])
                    return first

                def emit_t1():
                    T1_ps = big_ps.tile([128, EB, E], F32, tag="big")
                    for m in range(EB):
                        msl = slice(m * 128, (m + 1) * 128)
                        for ec in range(EB):
                            nc.tensor.matmul(
                                T1_ps[:, m, :],
                                X_sb[:, ec, msl],
                                wq_sb[:, ec, :],
                                start=ec == 0,
                                stop=ec == EB - 1,
                            )
                    T1_sb = t1_pool.tile([128, EB, E], BF16, tag="t1")
                    nc.vector.tensor_copy(T1_sb[:, 0, :], T1_ps[:, 0, :])
                    nc.scalar.activation(T1_sb[:, 1, :], T1_ps[:, 1, :], COPY)
                    nc.vector.tensor_copy(T1_sb[:, 2, :], T1_ps[:, 2, :])
                    nc.scalar.activation(T1_sb[:, 3, :], T1_ps[:, 3, :], COPY)
                    return T1_sb

                if u == 0 and hl == 0:
                    # first head: nothing to cover before T1, and P's psum
                    # slots want the G1 casts done -- T1's matmuls cover that
                    T1_sb = emit_t1()
                    p_first = emit_p_group(0, False)
                    emit_p_group(1, True)
                    emit_p_group(2, True)
                    emit_p_group(3, False)
                else:
                    # first half of P covers the previous head's exp wait
                    p_first = emit_p_group(0, False)
                    emit_p_group(1, True)
                    T1_sb = emit_t1()
                    # second half of P covers the T1 casts
                    emit_p_group(2, True)
                    emit_p_group(3, False)
                gate_hist.append(p_first.ins)
                emit_pending_s()

                # --- attT = Wk_h^T @ T1 + rk1 rank-2 update ---
                attT_ps = big_ps.tile([128, EB, E], F32, tag="big")
                for fb in range(EB):
                    fsl = slice(fb * 128, (fb + 1) * 128)
                    for ec in range(EB):
                        nc.tensor.matmul(
                            attT_ps[:, fb, :],
                            wk_sb[:, ec, fsl],
                            T1_sb[:, ec, :],
                            start=ec == 0,
                            stop=False,
                        )
                    nc.tensor.matmul(
                        attT_ps[:, fb, :],
                        rk1_sb[0:2, u, hl, 0, fsl],
                        rk1_sb[0:2, u, hl, 1, :],
                        start=False,
                        stop=True,
                    )

                # --- exp (softmax numerator, transposed layout) ---
                A_sb = a_pool.tile([128, EB, E], BF16, tag="a")
                for fb in range(EB):
                    nc.scalar.activation(
                        out=A_sb[:, fb, :],
                        in_=attT_ps[:, fb, :],
                        func=EXP,
                        scale=SCALE,
                    )
                A_tiles.append(A_sb)
                pending_s = (A_sb, R_tiles)

            # --- U_h = A_h^T @ P_h ; out = U0*r0 + U1*r1 ---
            out_tiles = [
                o_pool.tile([128, E], F32, tag="o", name=f"ot{u}_{i}")
                for i in range(EB)
            ]
            for hl in range(2):
                if hl == 1:
                    emit_pending_s()  # s of this unit's second head
                for eb in range(EB):
                    esl = slice(eb * 128, (eb + 1) * 128)
                    u_tile = (u_ps if eb % 2 == 0 else mm_ps).tile(
                        [128, E], F32, tag="u" if eb % 2 == 0 else "mm",
                        name=f"ut{hl}_{eb}",
                    )
                    for fc in range(EB):
                        nc.tensor.matmul(
                            u_tile[:],
                            A_tiles[hl][:, fc, esl],
                            P_sbs[hl][:, fc, :],
                            start=fc == 0,
                            stop=fc == EB - 1,
                        )
                    if hl == 0:
                        nc.vector.tensor_scalar_mul(
                            out_tiles[eb][:], u_tile[:], R_tiles[0][:, eb, 0:1]
                        )
                    else:
                        nc.vector.scalar_tensor_tensor(
                            out_tiles[eb][:],
                            u_tile[:],
                            R_tiles[1][:, eb, 0:1],
                            out_tiles[eb][:],
                            op0=mybir.AluOpType.mult,
                            op1=mybir.AluOpType.add,
                        )
                        nc.sync.dma_start(
                            out=out_d.ap()[u, eb * 128 : (eb + 1) * 128, :],
                            in_=out_tiles[eb][:],
                        )

    nc.compile()
    return nc


def _get_nc():
    global _CACHED_NC
    if _CACHED_NC is None:
        _CACHED_NC = build_nc()
    return _CACHED_NC


def make_in_maps(x, Wq, bq, Wk, bk, Wv, bv, Wp, bp):
    x = np.asarray(x, np.float32)
    Wq, Wk, Wv, Wp = (np.asarray(a, np.float32) for a in (Wq, Wk, Wv, Wp))
    bq, bk, bv, bp = (np.asarray(a, np.float32) for a in (bq, bk, bv, bp))

    def b16(a):
        return np.ascontiguousarray(a.astype(NP_BF16))

    wp_arr = b16(np.stack([Wp[:N], Wp[N:]]))
    swp = np.stack([Wp[:N].sum(0), Wp[N:].sum(0)])  # (2, E)
    in_maps = []
    for c in range(8):
        b = c // 2
        rs = [2 * (c % 2), 2 * (c % 2) + 1]
        heads = [[2 * r + hl for hl in range(2)] for r in rs]
        xs = x[b].sum(0)

        def tile_w(Wm, h):
            # (E, E) -> [p, t, e] with row t*128+p on partition p
            return Wm[:, h::H].reshape(EB, 128, E).transpose(1, 0, 2)

        wq_arr = b16(np.stack([[tile_w(Wq, h) for h in hu] for hu in heads]))
        wk_arr = b16(np.stack([[tile_w(Wk, h) for h in hu] for hu in heads]))
        wv_arr = b16(np.stack([[tile_w(Wv, h) for h in hu] for hu in heads]))
        # pb2[part, u, hl, role, :]: rank-2 P bias update:
        #   part 0: bv (lhsT) / swp_hl (rhs);  part 1: ones (lhsT) / bp/2
        pb2 = np.empty((2, 2, 2, 2, E), np.float32)
        # rk1[part, u, hl, role, :]: merged rank-2 bias update of attT:
        #   part 0: (Wk^T xs) (x) bq ; part 1: bk (x) (Wq^T xs + N bq)
        rk1 = np.empty((2, 2, 2, 2, E), np.float32)
        for iu, hu in enumerate(heads):
            for ihl, h in enumerate(hu):
                pb2[0, iu, ihl, 0] = bv[h::H]
                pb2[0, iu, ihl, 1] = swp[ihl]
                pb2[1, iu, ihl, 0] = 1.0
                pb2[1, iu, ihl, 1] = 0.5 * bp
                rk1[0, iu, ihl, 0] = Wk[:, h::H].T @ xs
                rk1[0, iu, ihl, 1] = bq[h::H]
                rk1[1, iu, ihl, 0] = bk[h::H]
                rk1[1, iu, ihl, 1] = Wq[:, h::H].T @ xs + np.float32(N) * bq[h::H]
        in_maps.append(
            {
                "xn": b16(x[b]),
                "wp": wp_arr,
                "wq": wq_arr,
                "wk": wk_arr,
                "wv": wv_arr,
                "pb2": b16(pb2),
                "rk1": b16(rk1),
            }
        )
    return in_maps


def assemble_out(results):
    out = np.empty((B, N, E), np.float32)
    for c in range(8):
        b = c // 2
        for ui in range(2):
            r = 2 * (c % 2) + ui
            out[b, r::4, :] = results[c]["out"][ui]
    return out


def run(inputs, trace=False, **spmd_kwargs):
    """Full pipeline; returns (output, BassKernelResults)."""
    nc = _get_nc()
    in_maps = make_in_maps(**inputs)
    res = run_bass_kernel_spmd(
        nc, in_maps, core_ids=list(range(8)), trace=trace, **spmd_kwargs
    )
    return assemble_out(res.results), res


def kernel(**inputs):
    out, _ = run(inputs)
    return out
